# revision 6
# baseline (speedup 1.0000x reference)
"""Dense2DSpatialTransformer (bilinear warp, N(0,1) flow) on 8 TRN2 cores.

V4 design. The per-pixel 2D gather runs as one-hot copy_predicated selects,
with every select element carrying TWO fp16 pixels packed in one int32:

  * Phase 0 builds an int32 "HP" image in DRAM directly from the input
    (element c = packed fp16 pair (I[h,c], I[h,c+1])), with replicate
    padding done in-SBUF (cols) and via 1-row DRAM copies (rows).  An fp32
    replicate-padded image is built in parallel for the exact sparse fixup.
  * Dense per tile: 9 candidate-row HP planes live in ONE SBUF tile; the
    8-way horizontal one-hot runs as 7 wide 3D-AP copy_predicated ops
    (mask broadcast over the row dim).  The two vertical one-hots (floor
    row N and ceil row S) are fused into one [128,2,F] select chain.
  * Masks/fractions from an fp16 flow copy via tensor_scalar (4x DVE mode);
    the DVE float->int convert rounds, so floors use a +7.5 bias.
  * Fractions + bilinear lerp are offloaded to the idle GPSIMD engine;
    fp32->fp16 flow converts to the Activation engine.  Output is fp16,
    widened on host.
  * Outliers (|disp| beyond the window, ~1e-4) are computed exactly in
    fp32 via indirect-DMA gathers (hoisted before the dense loop) and
    scattered over the dense result at the end.
"""
import sys

for _p in ("/opt/trn_rl_repo", "/opt/trn_rl_repo/concourse",
           "/root/.axon_site/_ro/trn_rl_repo"):
    if _p not in sys.path:
        sys.path.insert(0, _p)

import numpy as np

import concourse.bass as bass
import concourse.bacc as bacc
import concourse.mybir as mybir
import concourse.tile as tile
from concourse.bass import IndirectOffsetOnAxis
from concourse.bass_utils import run_bass_kernel_spmd

f32 = np.float32
FP = mybir.dt.float32
FP16 = mybir.dt.float16
I32 = mybir.dt.int32
I16 = mybir.dt.int16

B, H, W = 16, 1024, 1024
NCORES = 8
BPC = B // NCORES           # images per core
PAD = 8
PP = H + 2 * PAD            # padded image side (1040)
S_LO, S_HI = -4, 3          # dense integer-shift window (per axis)
F = 512                     # free-dim tile width
NROW = H // 128             # row blocks per image
NCOL = W // F               # col chunks per image
HW = H * W
OUT_TAIL = 128              # scratch tail for fixup padding writes
INIT = -1                   # shift covered by the one-hot init copy
HPW = 528                   # per-plane stride in the 9-plane HP mega-tile

AL = mybir.AluOpType


def _build_program(nout):
    nc = bacc.Bacc("TRN2", target_bir_lowering=False, debug=False,
                   enable_asserts=False, num_devices=NCORES)

    img_d = nc.dram_tensor("img", [BPC, H, W], FP, kind="ExternalInput")
    flow_d = nc.dram_tensor("flow", [BPC * 2 * HW], FP, kind="ExternalInput")
    opos_d = nc.dram_tensor("opos", [nout], I32, kind="ExternalInput")
    odh_d = nc.dram_tensor("odh", [nout], I32, kind="ExternalInput")
    odw_d = nc.dram_tensor("odw", [nout], I32, kind="ExternalInput")
    oh_d = nc.dram_tensor("oh", [nout], FP, kind="ExternalInput")
    oh1_d = nc.dram_tensor("oh1", [nout], FP, kind="ExternalInput")
    ow_d = nc.dram_tensor("ow", [nout], FP, kind="ExternalInput")
    ow1_d = nc.dram_tensor("ow1", [nout], FP, kind="ExternalInput")
    obase_d = nc.dram_tensor("obase", [nout], FP, kind="ExternalInput")
    ppad_d = nc.dram_tensor("ppad", [BPC * PP * PP], FP, kind="Internal")
    hp_d = nc.dram_tensor("hp", [BPC, PP, PP], I32, kind="Internal")
    out_d = nc.dram_tensor("out", [BPC * HW + OUT_TAIL], FP16,
                           kind="ExternalOutput")

    img = img_d.ap()
    flowf = flow_d.ap()
    flow4 = flowf.rearrange("(b c h w) -> b c h w", b=BPC, c=2, h=H, w=W)
    ppf = ppad_d.ap()
    pp3 = ppf.rearrange("(b h w) -> b h w", b=BPC, h=PP, w=PP)
    hp3 = hp_d.ap()
    outf = out_d.ap()
    out3 = outf[0:BPC * HW].rearrange("(b h w) -> b h w", b=BPC, h=H, w=W)

    v = nc.vector
    g = nc.gpsimd

    with tile.TileContext(nc) as tc:
        # ---- phase 0a: HP packed-pair fp16 image, directly from img ----
        # hp[b, 8+h, c] = int32( fp16(I[h,c-8]), fp16(I[h,c-8+1]) ), with
        # replicate padding: cols done in-SBUF, rows via 1-row DRAM copies.
        with tc.tile_pool(name="hpb", bufs=2) as hb:
            for b in range(BPC):
                for blk in range(NROW):
                    rs = 128 * blk
                    p32 = hb.tile([128, PP], FP, tag="p32")
                    nc.sync.dma_start(out=p32[:, PAD:PAD + W],
                                      in_=img[b, rs:rs + 128, :])
                    v.tensor_copy(
                        out=p32[:, 3:PAD],
                        in_=p32[:, PAD:PAD + 1].broadcast_to((128, PAD - 3)))
                    v.tensor_copy(
                        out=p32[:, PAD + W:PAD + W + 6],
                        in_=p32[:, PAD + W - 1:PAD + W].broadcast_to((128, 6)))
                    hpt = hb.tile([128, PP], I32, tag="hpt")
                    v16 = hpt[:].bitcast(FP16).rearrange(
                        "p (c two) -> p c two", two=2)
                    v.tensor_copy(out=v16[:, 3:1037, 0:1],
                                  in_=p32[:, 3:1037].unsqueeze(-1))
                    nc.scalar.copy(out=v16[:, 3:1037, 1:2],
                                   in_=p32[:, 4:1038].unsqueeze(-1))
                    nc.sync.dma_start(out=hp3[b, PAD + rs:PAD + rs + 128,
                                              3:1037],
                                      in_=hpt[:, 3:1037])
                # replicate packed edge rows (row 8 -> 3..7, 1031 -> 1032..36)
                for k in range(3, PAD):
                    nc.sync.dma_start(out=hp3[b, k:k + 1, 3:1037],
                                      in_=hp3[b, PAD:PAD + 1, 3:1037])
                for k in range(PAD + H, PAD + H + 5):
                    nc.sync.dma_start(out=hp3[b, k:k + 1, 3:1037],
                                      in_=hp3[b, PAD + H - 1:PAD + H, 3:1037])

        # ---- phase 0b: fp32 replicate-padded image (fixup gathers) ----
        for b in range(BPC):
            nc.sync.dma_start(out=pp3[b, PAD:PAD + H, PAD:PAD + W],
                              in_=img[b])
            for k in range(PAD):
                nc.sync.dma_start(out=pp3[b, k:k + 1, PAD:PAD + W],
                                  in_=img[b, 0:1, :])
                nc.sync.dma_start(
                    out=pp3[b, PAD + H + k:PAD + H + k + 1, PAD:PAD + W],
                    in_=img[b, H - 1:H, :])
        rblocks = []
        rs = 0
        while rs < PP:
            rn = min(128, PP - rs)
            rblocks.append((rs, rn))
            rs += rn
        with tc.tile_pool(name="pad", bufs=2) as pd:
            for b in range(BPC):
                for (rs, rn) in rblocks:
                    cl = pd.tile([128, 1], FP, tag="cl")
                    nc.sync.dma_start(out=cl[0:rn],
                                      in_=pp3[b, rs:rs + rn, PAD:PAD + 1])
                    ct = pd.tile([128, PAD], FP, tag="ct")
                    v.tensor_copy(out=ct[0:rn],
                                  in_=cl[0:rn, 0:1].broadcast_to((rn, PAD)))
                    nc.sync.dma_start(out=pp3[b, rs:rs + rn, 0:PAD],
                                      in_=ct[0:rn])
                    cr = pd.tile([128, 1], FP, tag="cr")
                    nc.sync.dma_start(
                        out=cr[0:rn],
                        in_=pp3[b, rs:rs + rn, PAD + W - 1:PAD + W])
                    cu = pd.tile([128, PAD], FP, tag="cu")
                    v.tensor_copy(out=cu[0:rn],
                                  in_=cr[0:rn, 0:1].broadcast_to((rn, PAD)))
                    nc.sync.dma_start(
                        out=pp3[b, rs:rs + rn, PAD + W:PAD + W + PAD],
                        in_=cu[0:rn])

        TS = [t for t in range(S_LO, S_HI + 1) if t != INIT]
        NCH = nout // 128

        # ---- sparse fixup, compute part (hoisted before the dense loop) ----
        with tc.tile_pool(name="fix", bufs=1) as fx:
            def load_aux(d, dt, name):
                t = fx.tile([128, NCH], dt, tag=name)
                nc.sync.dma_start(
                    out=t[:],
                    in_=d.ap().rearrange("(p f) -> p f", p=128))
                return t

            opos_s = load_aux(opos_d, I32, "opos")
            odh_s = load_aux(odh_d, I32, "odh")
            odw_s = load_aux(odw_d, I32, "odw")
            oh_s = load_aux(oh_d, FP, "oh")
            oh1_s = load_aux(oh1_d, FP, "oh1")
            ow_s = load_aux(ow_d, FP, "ow")
            ow1_s = load_aux(ow1_d, FP, "ow1")
            obase_s = load_aux(obase_d, FP, "obase")

            dhv = fx.tile([128, NCH], FP, tag="dhv")
            dwv = fx.tile([128, NCH], FP, tag="dwv")
            for c in range(NCH):
                g.indirect_dma_start(
                    out=dhv[:, c:c + 1], out_offset=None,
                    in_=flowf[:, None],
                    in_offset=IndirectOffsetOnAxis(
                        ap=odh_s[:, c:c + 1], axis=0))
                g.indirect_dma_start(
                    out=dwv[:, c:c + 1], out_offset=None,
                    in_=flowf[:, None],
                    in_offset=IndirectOffsetOnAxis(
                        ap=odw_s[:, c:c + 1], axis=0))

            def fields(dv, hb_, hb1, pfx):
                yt = fx.tile([128, NCH], FP, tag=f"{pfx}y")
                v.tensor_tensor(out=yt[:], in0=dv[:], in1=hb_[:], op=AL.add)
                v.tensor_scalar(out=yt[:], in0=yt[:], scalar1=1.0,
                                scalar2=None, op0=AL.add)
                Rt = fx.tile([128, NCH], FP, tag=f"{pfx}R")
                v.tensor_tensor(out=Rt[:], in0=yt[:], in1=hb1[:],
                                op=AL.subtract)
                St = fx.tile([128, NCH], FP, tag=f"{pfx}S")
                gt = fx.tile([128, NCH], FP, tag=f"{pfx}g")
                v.tensor_scalar(out=St[:], in0=Rt[:], scalar1=-6.0,
                                scalar2=None, op0=AL.is_ge)
                for s in range(-5, 7):
                    v.tensor_scalar(out=gt[:], in0=Rt[:], scalar1=float(s),
                                    scalar2=None, op0=AL.is_ge)
                    v.tensor_tensor(out=St[:], in0=St[:], in1=gt[:],
                                    op=AL.add)
                dt_ = fx.tile([128, NCH], FP, tag=f"{pfx}d")
                v.tensor_scalar(out=dt_[:], in0=St[:], scalar1=-6.0,
                                scalar2=None, op0=AL.add)
                v.tensor_tensor(out=dt_[:], in0=dt_[:], in1=Rt[:],
                                op=AL.subtract)
                return yt, dt_

            yv, dhw = fields(dhv, oh_s, oh1_s, "fh")
            ywv, dww = fields(dwv, ow_s, ow1_s, "fw")

            rowp = fx.tile([128, NCH], FP, tag="rowp")
            v.tensor_tensor(out=rowp[:], in0=yv[:], in1=dhw[:], op=AL.add)
            v.tensor_scalar(out=rowp[:], in0=rowp[:], scalar1=6.0,
                            scalar2=float(PP), op0=AL.add, op1=AL.mult)
            colp = fx.tile([128, NCH], FP, tag="colp")
            v.tensor_tensor(out=colp[:], in0=ywv[:], in1=dww[:], op=AL.add)
            v.tensor_scalar(out=colp[:], in0=colp[:], scalar1=6.0,
                            scalar2=None, op0=AL.add)
            af = fx.tile([128, NCH], FP, tag="af")
            v.tensor_tensor(out=af[:], in0=rowp[:], in1=colp[:], op=AL.add)
            v.tensor_tensor(out=af[:], in0=af[:], in1=obase_s[:], op=AL.add)

            vals = {}
            afo = fx.tile([128, NCH], FP, tag="afo")
            for (cn, doff) in (("v00", 0.0), ("v10", 1.0),
                               ("v01", float(PP)), ("v11", float(PP + 1))):
                ai = fx.tile([128, NCH], I32, tag=f"ai{cn}")
                if doff == 0.0:
                    v.tensor_copy(out=ai[:], in_=af[:])
                else:
                    v.tensor_scalar(out=afo[:], in0=af[:], scalar1=doff,
                                    scalar2=None, op0=AL.add)
                    v.tensor_copy(out=ai[:], in_=afo[:])
                vt = fx.tile([128, NCH], FP, tag=cn)
                for c in range(NCH):
                    g.indirect_dma_start(
                        out=vt[:, c:c + 1], out_offset=None,
                        in_=ppf[:, None],
                        in_offset=IndirectOffsetOnAxis(
                            ap=ai[:, c:c + 1], axis=0))
                vals[cn] = vt

            omw_f = fx.tile([128, NCH], FP, tag="omwf")
            v.tensor_scalar(out=omw_f[:], in0=dww[:], scalar1=-1.0,
                            scalar2=1.0, op0=AL.mult, op1=AL.add)
            omh_f = fx.tile([128, NCH], FP, tag="omhf")
            v.tensor_scalar(out=omh_f[:], in0=dhw[:], scalar1=-1.0,
                            scalar2=1.0, op0=AL.mult, op1=AL.add)
            wt = fx.tile([128, NCH], FP, tag="wtf")
            accf = fx.tile([128, NCH], FP, tag="accf")
            t3f = fx.tile([128, NCH], FP, tag="t3f")
            v.tensor_tensor(out=wt[:], in0=dhw[:], in1=dww[:], op=AL.mult)
            v.tensor_tensor(out=accf[:], in0=vals["v00"][:], in1=wt[:],
                            op=AL.mult)
            v.tensor_tensor(out=wt[:], in0=dhw[:], in1=omw_f[:], op=AL.mult)
            v.tensor_tensor(out=t3f[:], in0=vals["v10"][:], in1=wt[:],
                            op=AL.mult)
            v.tensor_tensor(out=accf[:], in0=accf[:], in1=t3f[:], op=AL.add)
            v.tensor_tensor(out=wt[:], in0=omh_f[:], in1=dww[:], op=AL.mult)
            v.tensor_tensor(out=t3f[:], in0=vals["v01"][:], in1=wt[:],
                            op=AL.mult)
            v.tensor_tensor(out=accf[:], in0=accf[:], in1=t3f[:], op=AL.add)
            v.tensor_tensor(out=wt[:], in0=omw_f[:], in1=omh_f[:],
                            op=AL.mult)
            v.tensor_tensor(out=t3f[:], in0=vals["v11"][:], in1=wt[:],
                            op=AL.mult)
            v.tensor_tensor(out=accf[:], in0=accf[:], in1=t3f[:], op=AL.add)
            acc16 = fx.tile([128, NCH], FP16, tag="acc16")
            v.tensor_copy(out=acc16[:], in_=accf[:])

            # ---- dense tiles ----
            with tc.tile_pool(name="wk", bufs=2) as wk:
                for b in range(BPC):
                    for i in range(NROW):
                        for j in range(NCOL):
                            r0 = 128 * i
                            w0 = F * j
                            hpa = wk.tile([128, 9 * HPW], I32, tag="hpa")
                            hp9 = hpa[:].rearrange("p (r c) -> p r c", r=9)
                            for r in range(9):
                                nc.sync.dma_start(
                                    out=hp9[:, r:r + 1, 0:F + 8],
                                    in_=hp3[b, PAD + r0 - 4 + r:
                                            PAD + r0 - 4 + r + 128,
                                            PAD + w0 - 4:PAD + w0 + F + 4
                                            ].unsqueeze(1))
                            f32h = wk.tile([128, F], FP, tag="f32h")
                            nc.sync.dma_start(
                                out=f32h[:],
                                in_=flow4[b, 0, r0:r0 + 128, w0:w0 + F])
                            f32w = wk.tile([128, F], FP, tag="f32w")
                            nc.sync.dma_start(
                                out=f32w[:],
                                in_=flow4[b, 1, r0:r0 + 128, w0:w0 + F])
                            dh16 = wk.tile([128, F], FP16, tag="dh16")
                            nc.scalar.copy(out=dh16[:], in_=f32h[:])
                            dw16 = wk.tile([128, F], FP16, tag="dw16")
                            nc.scalar.copy(out=dw16[:], in_=f32w[:])

                            # floors (DVE float->int convert ROUNDS: +7.5)
                            nw8 = wk.tile([128, F], I16, tag="nw8")
                            v.tensor_scalar(out=nw8[:], in0=dw16[:],
                                            scalar1=7.5, scalar2=None,
                                            op0=AL.add)
                            kh8 = wk.tile([128, F], I16, tag="kh8")
                            v.tensor_scalar(out=kh8[:], in0=dh16[:],
                                            scalar1=7.5, scalar2=None,
                                            op0=AL.add)

                            mw = {}
                            for t in TS:
                                m = wk.tile([128, F], I16, tag=f"mw{t}")
                                v.tensor_scalar(out=m[:], in0=nw8[:],
                                                scalar1=float(t + 8),
                                                scalar2=None,
                                                op0=AL.is_equal)
                                mw[t] = m
                            mv = {}
                            for s in TS:
                                m = wk.tile([128, F], I16, tag=f"mv{s}")
                                v.tensor_scalar(out=m[:], in0=kh8[:],
                                                scalar1=float(s + 8),
                                                scalar2=None,
                                                op0=AL.is_equal)
                                mv[s] = m

                            # fractions (GPSIMD)
                            flw = wk.tile([128, F], FP16, tag="flw")
                            g.tensor_scalar(out=flw[:], in0=nw8[:],
                                            scalar1=8.0, scalar2=None,
                                            op0=AL.subtract)
                            gam = wk.tile([128, F], FP16, tag="gam")
                            g.tensor_tensor(out=gam[:], in0=dw16[:],
                                            in1=flw[:], op=AL.subtract)
                            flh = wk.tile([128, F], FP16, tag="flh")
                            g.tensor_scalar(out=flh[:], in0=kh8[:],
                                            scalar1=8.0, scalar2=None,
                                            op0=AL.subtract)
                            alp = wk.tile([128, F], FP16, tag="alp")
                            g.tensor_tensor(out=alp[:], in0=dh16[:],
                                            in1=flh[:], op=AL.subtract)

                            # horizontal one-hot: all 9 rows in one 3D op
                            cb = wk.tile([128, 9 * F], I32, tag="cb")
                            cb9 = cb[:].rearrange("p (r c) -> p r c", r=9)
                            v.tensor_copy(out=cb9[:],
                                          in_=hp9[:, :, INIT + 4:INIT + 4 + F])
                            for t in TS:
                                mb = mw[t][:].unsqueeze(1).broadcast_to(
                                    (128, 9, F))
                                v.copy_predicated(
                                    out=cb9[:], mask=mb,
                                    data=hp9[:, :, t + 4:t + 4 + F])

                            # vertical one-hot, N and S fused: [128, 2, F]
                            ns = wk.tile([128, 2 * F], I32, tag="ns")
                            ns2 = ns[:].rearrange("p (r c) -> p r c", r=2)
                            v.tensor_copy(out=ns2[:],
                                          in_=cb9[:, INIT + 4:INIT + 6, :])
                            for s in TS:
                                mb = mv[s][:].unsqueeze(1).broadcast_to(
                                    (128, 2, F))
                                v.copy_predicated(
                                    out=ns2[:], mask=mb,
                                    data=cb9[:, s + 4:s + 6, :])

                            # unpack + bilinear lerp (GPSIMD)
                            nsv = ns[:].bitcast(FP16).rearrange(
                                "p (r c two) -> p r c two", r=2, two=2)
                            nw_v = nsv[:, 0, :, 0:1].squeeze(-1)
                            ne_v = nsv[:, 0, :, 1:2].squeeze(-1)
                            sw_v = nsv[:, 1, :, 0:1].squeeze(-1)
                            se_v = nsv[:, 1, :, 1:2].squeeze(-1)

                            dn = wk.tile([128, F], FP16, tag="dn")
                            g.tensor_tensor(out=dn[:], in0=ne_v, in1=nw_v,
                                            op=AL.subtract)
                            t1 = wk.tile([128, F], FP16, tag="t1")
                            g.tensor_tensor(out=t1[:], in0=gam[:], in1=dn[:],
                                            op=AL.mult)
                            hn = wk.tile([128, F], FP16, tag="hn")
                            g.tensor_tensor(out=hn[:], in0=nw_v, in1=t1[:],
                                            op=AL.add)
                            ds = wk.tile([128, F], FP16, tag="ds")
                            g.tensor_tensor(out=ds[:], in0=se_v, in1=sw_v,
                                            op=AL.subtract)
                            t2 = wk.tile([128, F], FP16, tag="t2")
                            g.tensor_tensor(out=t2[:], in0=gam[:], in1=ds[:],
                                            op=AL.mult)
                            hs = wk.tile([128, F], FP16, tag="hs")
                            g.tensor_tensor(out=hs[:], in0=sw_v, in1=t2[:],
                                            op=AL.add)
                            dv_ = wk.tile([128, F], FP16, tag="dv")
                            g.tensor_tensor(out=dv_[:], in0=hs[:], in1=hn[:],
                                            op=AL.subtract)
                            t3 = wk.tile([128, F], FP16, tag="t3")
                            g.tensor_tensor(out=t3[:], in0=alp[:], in1=dv_[:],
                                            op=AL.mult)
                            o16 = wk.tile([128, F], FP16, tag="o16")
                            g.tensor_tensor(out=o16[:], in0=hn[:], in1=t3[:],
                                            op=AL.add)
                            nc.sync.dma_start(
                                out=out3[b, r0:r0 + 128, w0:w0 + F],
                                in_=o16[:])

            # ---- fixup scatter (after dense stores) ----
            for c in range(NCH):
                g.indirect_dma_start(
                    out=outf[:, None],
                    out_offset=IndirectOffsetOnAxis(
                        ap=opos_s[:, c:c + 1], axis=0),
                    in_=acc16[:, c:c + 1], in_offset=None)

    nc.compile()
    return nc


_PROGRAM_CACHE = {}


def _get_program(nout):
    if nout not in _PROGRAM_CACHE:
        _PROGRAM_CACHE[nout] = _build_program(nout)
    return _PROGRAM_CACHE[nout]


def _host_inlier_mask(d):
    """Mirror the device fp16 floor: rint(fp32(fp16(d)) + 7.5) in [4, 11]."""
    d16 = d.astype(np.float16).astype(np.float32)
    n8 = np.rint(d16 + np.float32(7.5)).astype(np.int32)
    return (n8 >= S_LO + 8) & (n8 <= S_HI + 8)


def _host_metadata(dH, dW):
    """Outlier positions for one image under the device dense criterion."""
    inl = _host_inlier_mask(dH) & _host_inlier_mask(dW)
    oy, ox = np.where(~inl)
    return oy.astype(np.int64), ox.astype(np.int64)


def _prepare(input1, input2):
    """Build (or fetch) the program and the per-core input maps."""
    input1 = np.asarray(input1)
    input2 = np.asarray(input2)
    assert input1.shape == (B, 1, H, W) and input2.shape == (B, 2, H, W)

    metas = []
    max_n = 1
    for c in range(NCORES):
        rows = []
        for bl in range(BPC):
            bglob = c * BPC + bl
            oy, ox = _host_metadata(input2[bglob, 0], input2[bglob, 1])
            rows.append((bl, oy, ox))
        n = sum(len(oy) for _, oy, _ in rows)
        max_n = max(max_n, n)
        metas.append(rows)
    nout = max(128, ((max_n + 127) // 128) * 128)

    nc = _get_program(nout)

    in_maps = []
    for c in range(NCORES):
        imgs = input1[c * BPC:(c + 1) * BPC, 0]
        flow = input2[c * BPC:(c + 1) * BPC]
        opos = np.full(nout, BPC * HW, np.int32)
        odh = np.zeros(nout, np.int32)
        odw = np.full(nout, HW, np.int32)
        oh = np.zeros(nout, f32)
        ow = np.zeros(nout, f32)
        obase = np.zeros(nout, f32)
        k = 0
        for bl, oy, ox in metas[c]:
            n = len(oy)
            opos[k:k + n] = (bl * HW + oy * W + ox).astype(np.int32)
            odh[k:k + n] = (bl * 2 * HW + oy * W + ox).astype(np.int32)
            odw[k:k + n] = (bl * 2 * HW + HW + oy * W + ox).astype(np.int32)
            oh[k:k + n] = oy.astype(f32)
            ow[k:k + n] = ox.astype(f32)
            obase[k:k + n] = f32(bl * PP * PP)
            k += n
        in_maps.append({
            "img": np.ascontiguousarray(imgs),
            "flow": np.ascontiguousarray(flow.reshape(-1)),
            "opos": opos, "odh": odh, "odw": odw,
            "oh": oh, "oh1": (oh + f32(1.0)).astype(f32),
            "ow": ow, "ow1": (ow + f32(1.0)).astype(f32),
            "obase": obase,
        })

    return nc, in_maps


def _assemble(results):
    out = np.empty((B, 1, H, W), f32)
    for c in range(NCORES):
        o = results[c]["out"][:BPC * HW].reshape(BPC, H, W)
        out[c * BPC:(c + 1) * BPC, 0] = o.astype(f32)
    return out


def kernel(input1, input2):
    nc, in_maps = _prepare(input1, input2)
    res = run_bass_kernel_spmd(nc, in_maps, core_ids=list(range(NCORES)))
    return _assemble(res.results)


# revision 7
# speedup vs baseline: 1.0094x; 1.0094x over previous
"""Dense2DSpatialTransformer (bilinear warp, N(0,1) flow) on 8 TRN2 cores.

V4 design. The per-pixel 2D gather runs as one-hot copy_predicated selects,
with every select element carrying TWO fp16 pixels packed in one int32:

  * Phase 0 builds an int32 "HP" image in DRAM directly from the input
    (element c = packed fp16 pair (I[h,c], I[h,c+1])), with replicate
    padding done in-SBUF (cols) and via 1-row DRAM copies (rows).  An fp32
    replicate-padded image is built in parallel for the exact sparse fixup.
  * Dense per tile: 9 candidate-row HP planes live in ONE SBUF tile; the
    8-way horizontal one-hot runs as 7 wide 3D-AP copy_predicated ops
    (mask broadcast over the row dim).  The two vertical one-hots (floor
    row N and ceil row S) are fused into one [128,2,F] select chain.
  * Masks/fractions from an fp16 flow copy via tensor_scalar (4x DVE mode);
    the DVE float->int convert rounds, so floors use a +7.5 bias.
  * Fractions + bilinear lerp are offloaded to the idle GPSIMD engine;
    fp32->fp16 flow converts to the Activation engine.  Output is fp16,
    widened on host.
  * Outliers (|disp| beyond the window, ~1e-4) are computed exactly in
    fp32 via indirect-DMA gathers (hoisted before the dense loop) and
    scattered over the dense result at the end.
"""
import sys

for _p in ("/opt/trn_rl_repo", "/opt/trn_rl_repo/concourse",
           "/root/.axon_site/_ro/trn_rl_repo"):
    if _p not in sys.path:
        sys.path.insert(0, _p)

import numpy as np

import concourse.bass as bass
import concourse.bacc as bacc
import concourse.mybir as mybir
import concourse.tile as tile
from concourse.bass import IndirectOffsetOnAxis
from concourse.bass_utils import run_bass_kernel_spmd

f32 = np.float32
FP = mybir.dt.float32
FP16 = mybir.dt.float16
I32 = mybir.dt.int32
I16 = mybir.dt.int16

B, H, W = 16, 1024, 1024
NCORES = 8
BPC = B // NCORES           # images per core
PAD = 8
PP = H + 2 * PAD            # padded image side (1040)
S_LO, S_HI = -4, 3          # dense integer-shift window (per axis)
F = 512                     # free-dim tile width
NROW = H // 128             # row blocks per image
NCOL = W // F               # col chunks per image
HW = H * W
OUT_TAIL = 128              # scratch tail for fixup padding writes
INIT = -1                   # shift covered by the one-hot init copy
HPW = 528                   # per-plane stride in the 9-plane HP mega-tile

AL = mybir.AluOpType


def _build_program(nout):
    nc = bacc.Bacc("TRN2", target_bir_lowering=False, debug=False,
                   enable_asserts=False, num_devices=NCORES)

    img_d = nc.dram_tensor("img", [BPC, H, W], FP, kind="ExternalInput")
    flow_d = nc.dram_tensor("flow", [BPC * 2 * HW], FP, kind="ExternalInput")
    opos_d = nc.dram_tensor("opos", [nout], I32, kind="ExternalInput")
    odh_d = nc.dram_tensor("odh", [nout], I32, kind="ExternalInput")
    odw_d = nc.dram_tensor("odw", [nout], I32, kind="ExternalInput")
    oh_d = nc.dram_tensor("oh", [nout], FP, kind="ExternalInput")
    oh1_d = nc.dram_tensor("oh1", [nout], FP, kind="ExternalInput")
    ow_d = nc.dram_tensor("ow", [nout], FP, kind="ExternalInput")
    ow1_d = nc.dram_tensor("ow1", [nout], FP, kind="ExternalInput")
    obase_d = nc.dram_tensor("obase", [nout], FP, kind="ExternalInput")
    ppad_d = nc.dram_tensor("ppad", [BPC * PP * PP], FP, kind="Internal")
    hp_d = nc.dram_tensor("hp", [BPC, PP, PP], I32, kind="Internal")
    out_d = nc.dram_tensor("out", [BPC * HW + OUT_TAIL], FP16,
                           kind="ExternalOutput")

    img = img_d.ap()
    flowf = flow_d.ap()
    flow4 = flowf.rearrange("(b c h w) -> b c h w", b=BPC, c=2, h=H, w=W)
    ppf = ppad_d.ap()
    pp3 = ppf.rearrange("(b h w) -> b h w", b=BPC, h=PP, w=PP)
    hp3 = hp_d.ap()
    outf = out_d.ap()
    out3 = outf[0:BPC * HW].rearrange("(b h w) -> b h w", b=BPC, h=H, w=W)

    v = nc.vector
    g = nc.gpsimd

    with tile.TileContext(nc) as tc:
        # ---- phase 0a: HP packed-pair fp16 image, directly from img ----
        # hp[b, 8+h, c] = int32( fp16(I[h,c-8]), fp16(I[h,c-8+1]) ), with
        # replicate padding: cols done in-SBUF, rows via 1-row DRAM copies.
        with tc.tile_pool(name="hpb", bufs=2) as hb:
            for b in range(BPC):
                for blk in range(NROW):
                    rs = 128 * blk
                    p32 = hb.tile([128, PP], FP, tag="p32")
                    nc.sync.dma_start(out=p32[:, PAD:PAD + W],
                                      in_=img[b, rs:rs + 128, :])
                    v.tensor_copy(
                        out=p32[:, 3:PAD],
                        in_=p32[:, PAD:PAD + 1].broadcast_to((128, PAD - 3)))
                    v.tensor_copy(
                        out=p32[:, PAD + W:PAD + W + 6],
                        in_=p32[:, PAD + W - 1:PAD + W].broadcast_to((128, 6)))
                    hpt = hb.tile([128, PP], I32, tag="hpt")
                    v16 = hpt[:].bitcast(FP16).rearrange(
                        "p (c two) -> p c two", two=2)
                    v.tensor_copy(out=v16[:, 3:1037, 0:1],
                                  in_=p32[:, 3:1037].unsqueeze(-1))
                    nc.scalar.copy(out=v16[:, 3:1037, 1:2],
                                   in_=p32[:, 4:1038].unsqueeze(-1))
                    nc.sync.dma_start(out=hp3[b, PAD + rs:PAD + rs + 128,
                                              3:1037],
                                      in_=hpt[:, 3:1037])
                # replicate packed edge rows (row 8 -> 3..7, 1031 -> 1032..36)
                for k in range(3, PAD):
                    nc.sync.dma_start(out=hp3[b, k:k + 1, 3:1037],
                                      in_=hp3[b, PAD:PAD + 1, 3:1037])
                for k in range(PAD + H, PAD + H + 5):
                    nc.sync.dma_start(out=hp3[b, k:k + 1, 3:1037],
                                      in_=hp3[b, PAD + H - 1:PAD + H, 3:1037])

        # ---- phase 0b: fp32 replicate-padded image (fixup gathers) ----
        for b in range(BPC):
            nc.sync.dma_start(out=pp3[b, PAD:PAD + H, PAD:PAD + W],
                              in_=img[b])
            for k in range(PAD):
                nc.sync.dma_start(out=pp3[b, k:k + 1, PAD:PAD + W],
                                  in_=img[b, 0:1, :])
                nc.sync.dma_start(
                    out=pp3[b, PAD + H + k:PAD + H + k + 1, PAD:PAD + W],
                    in_=img[b, H - 1:H, :])
        rblocks = []
        rs = 0
        while rs < PP:
            rn = min(128, PP - rs)
            rblocks.append((rs, rn))
            rs += rn
        with tc.tile_pool(name="pad", bufs=2) as pd:
            for b in range(BPC):
                for (rs, rn) in rblocks:
                    cl = pd.tile([128, 1], FP, tag="cl")
                    nc.sync.dma_start(out=cl[0:rn],
                                      in_=pp3[b, rs:rs + rn, PAD:PAD + 1])
                    ct = pd.tile([128, PAD], FP, tag="ct")
                    v.tensor_copy(out=ct[0:rn],
                                  in_=cl[0:rn, 0:1].broadcast_to((rn, PAD)))
                    nc.sync.dma_start(out=pp3[b, rs:rs + rn, 0:PAD],
                                      in_=ct[0:rn])
                    cr = pd.tile([128, 1], FP, tag="cr")
                    nc.sync.dma_start(
                        out=cr[0:rn],
                        in_=pp3[b, rs:rs + rn, PAD + W - 1:PAD + W])
                    cu = pd.tile([128, PAD], FP, tag="cu")
                    v.tensor_copy(out=cu[0:rn],
                                  in_=cr[0:rn, 0:1].broadcast_to((rn, PAD)))
                    nc.sync.dma_start(
                        out=pp3[b, rs:rs + rn, PAD + W:PAD + W + PAD],
                        in_=cu[0:rn])

        TS = [t for t in range(S_LO, S_HI + 1) if t != INIT]
        NCH = nout // 128

        # ---- sparse fixup, compute part (hoisted before the dense loop) ----
        with tc.tile_pool(name="fix", bufs=1) as fx:
            def load_aux(d, dt, name):
                t = fx.tile([128, NCH], dt, tag=name)
                nc.sync.dma_start(
                    out=t[:],
                    in_=d.ap().rearrange("(p f) -> p f", p=128))
                return t

            opos_s = load_aux(opos_d, I32, "opos")
            odh_s = load_aux(odh_d, I32, "odh")
            odw_s = load_aux(odw_d, I32, "odw")
            oh_s = load_aux(oh_d, FP, "oh")
            oh1_s = load_aux(oh1_d, FP, "oh1")
            ow_s = load_aux(ow_d, FP, "ow")
            ow1_s = load_aux(ow1_d, FP, "ow1")
            obase_s = load_aux(obase_d, FP, "obase")

            dhv = fx.tile([128, NCH], FP, tag="dhv")
            dwv = fx.tile([128, NCH], FP, tag="dwv")
            for c in range(NCH):
                g.indirect_dma_start(
                    out=dhv[:, c:c + 1], out_offset=None,
                    in_=flowf[:, None],
                    in_offset=IndirectOffsetOnAxis(
                        ap=odh_s[:, c:c + 1], axis=0))
                g.indirect_dma_start(
                    out=dwv[:, c:c + 1], out_offset=None,
                    in_=flowf[:, None],
                    in_offset=IndirectOffsetOnAxis(
                        ap=odw_s[:, c:c + 1], axis=0))

            def fields(dv, hb_, hb1, pfx):
                yt = fx.tile([128, NCH], FP, tag=f"{pfx}y")
                v.tensor_tensor(out=yt[:], in0=dv[:], in1=hb_[:], op=AL.add)
                v.tensor_scalar(out=yt[:], in0=yt[:], scalar1=1.0,
                                scalar2=None, op0=AL.add)
                Rt = fx.tile([128, NCH], FP, tag=f"{pfx}R")
                v.tensor_tensor(out=Rt[:], in0=yt[:], in1=hb1[:],
                                op=AL.subtract)
                St = fx.tile([128, NCH], FP, tag=f"{pfx}S")
                gt = fx.tile([128, NCH], FP, tag=f"{pfx}g")
                v.tensor_scalar(out=St[:], in0=Rt[:], scalar1=-6.0,
                                scalar2=None, op0=AL.is_ge)
                for s in range(-5, 7):
                    v.tensor_scalar(out=gt[:], in0=Rt[:], scalar1=float(s),
                                    scalar2=None, op0=AL.is_ge)
                    v.tensor_tensor(out=St[:], in0=St[:], in1=gt[:],
                                    op=AL.add)
                dt_ = fx.tile([128, NCH], FP, tag=f"{pfx}d")
                v.tensor_scalar(out=dt_[:], in0=St[:], scalar1=-6.0,
                                scalar2=None, op0=AL.add)
                v.tensor_tensor(out=dt_[:], in0=dt_[:], in1=Rt[:],
                                op=AL.subtract)
                return yt, dt_

            yv, dhw = fields(dhv, oh_s, oh1_s, "fh")
            ywv, dww = fields(dwv, ow_s, ow1_s, "fw")

            rowp = fx.tile([128, NCH], FP, tag="rowp")
            v.tensor_tensor(out=rowp[:], in0=yv[:], in1=dhw[:], op=AL.add)
            v.tensor_scalar(out=rowp[:], in0=rowp[:], scalar1=6.0,
                            scalar2=float(PP), op0=AL.add, op1=AL.mult)
            colp = fx.tile([128, NCH], FP, tag="colp")
            v.tensor_tensor(out=colp[:], in0=ywv[:], in1=dww[:], op=AL.add)
            v.tensor_scalar(out=colp[:], in0=colp[:], scalar1=6.0,
                            scalar2=None, op0=AL.add)
            af = fx.tile([128, NCH], FP, tag="af")
            v.tensor_tensor(out=af[:], in0=rowp[:], in1=colp[:], op=AL.add)
            v.tensor_tensor(out=af[:], in0=af[:], in1=obase_s[:], op=AL.add)

            vals = {}
            afo = fx.tile([128, NCH], FP, tag="afo")
            for (cn, doff) in (("v00", 0.0), ("v10", 1.0),
                               ("v01", float(PP)), ("v11", float(PP + 1))):
                ai = fx.tile([128, NCH], I32, tag=f"ai{cn}")
                if doff == 0.0:
                    v.tensor_copy(out=ai[:], in_=af[:])
                else:
                    v.tensor_scalar(out=afo[:], in0=af[:], scalar1=doff,
                                    scalar2=None, op0=AL.add)
                    v.tensor_copy(out=ai[:], in_=afo[:])
                vt = fx.tile([128, NCH], FP, tag=cn)
                for c in range(NCH):
                    g.indirect_dma_start(
                        out=vt[:, c:c + 1], out_offset=None,
                        in_=ppf[:, None],
                        in_offset=IndirectOffsetOnAxis(
                            ap=ai[:, c:c + 1], axis=0))
                vals[cn] = vt

            omw_f = fx.tile([128, NCH], FP, tag="omwf")
            v.tensor_scalar(out=omw_f[:], in0=dww[:], scalar1=-1.0,
                            scalar2=1.0, op0=AL.mult, op1=AL.add)
            omh_f = fx.tile([128, NCH], FP, tag="omhf")
            v.tensor_scalar(out=omh_f[:], in0=dhw[:], scalar1=-1.0,
                            scalar2=1.0, op0=AL.mult, op1=AL.add)
            wt = fx.tile([128, NCH], FP, tag="wtf")
            accf = fx.tile([128, NCH], FP, tag="accf")
            t3f = fx.tile([128, NCH], FP, tag="t3f")
            v.tensor_tensor(out=wt[:], in0=dhw[:], in1=dww[:], op=AL.mult)
            v.tensor_tensor(out=accf[:], in0=vals["v00"][:], in1=wt[:],
                            op=AL.mult)
            v.tensor_tensor(out=wt[:], in0=dhw[:], in1=omw_f[:], op=AL.mult)
            v.tensor_tensor(out=t3f[:], in0=vals["v10"][:], in1=wt[:],
                            op=AL.mult)
            v.tensor_tensor(out=accf[:], in0=accf[:], in1=t3f[:], op=AL.add)
            v.tensor_tensor(out=wt[:], in0=omh_f[:], in1=dww[:], op=AL.mult)
            v.tensor_tensor(out=t3f[:], in0=vals["v01"][:], in1=wt[:],
                            op=AL.mult)
            v.tensor_tensor(out=accf[:], in0=accf[:], in1=t3f[:], op=AL.add)
            v.tensor_tensor(out=wt[:], in0=omw_f[:], in1=omh_f[:],
                            op=AL.mult)
            v.tensor_tensor(out=t3f[:], in0=vals["v11"][:], in1=wt[:],
                            op=AL.mult)
            v.tensor_tensor(out=accf[:], in0=accf[:], in1=t3f[:], op=AL.add)
            acc16 = fx.tile([128, NCH], FP16, tag="acc16")
            v.tensor_copy(out=acc16[:], in_=accf[:])

            # ---- dense tiles ----
            with tc.tile_pool(name="wk", bufs=2) as wk:
                for b in range(BPC):
                    for i in range(NROW):
                        for j in range(NCOL):
                            r0 = 128 * i
                            w0 = F * j
                            hpa = wk.tile([128, 9 * HPW], I32, tag="hpa")
                            hp9 = hpa[:].rearrange("p (r c) -> p r c", r=9)
                            for r in range(9):
                                nc.sync.dma_start(
                                    out=hp9[:, r:r + 1, 0:F + 8],
                                    in_=hp3[b, PAD + r0 - 4 + r:
                                            PAD + r0 - 4 + r + 128,
                                            PAD + w0 - 4:PAD + w0 + F + 4
                                            ].unsqueeze(1))
                            f32h = wk.tile([128, F], FP, tag="f32h")
                            nc.sync.dma_start(
                                out=f32h[:],
                                in_=flow4[b, 0, r0:r0 + 128, w0:w0 + F])
                            f32w = wk.tile([128, F], FP, tag="f32w")
                            nc.sync.dma_start(
                                out=f32w[:],
                                in_=flow4[b, 1, r0:r0 + 128, w0:w0 + F])
                            dh16 = wk.tile([128, F], FP16, tag="dh16")
                            nc.scalar.copy(out=dh16[:], in_=f32h[:])
                            dw16 = wk.tile([128, F], FP16, tag="dw16")
                            nc.scalar.copy(out=dw16[:], in_=f32w[:])

                            # floors (DVE float->int convert ROUNDS: +7.5)
                            nw8 = wk.tile([128, F], I16, tag="nw8")
                            v.tensor_scalar(out=nw8[:], in0=dw16[:],
                                            scalar1=7.5, scalar2=None,
                                            op0=AL.add)
                            kh8 = wk.tile([128, F], I16, tag="kh8")
                            v.tensor_scalar(out=kh8[:], in0=dh16[:],
                                            scalar1=7.5, scalar2=None,
                                            op0=AL.add)

                            mw = {}
                            for t in TS:
                                m = wk.tile([128, F], I16, tag=f"mw{t}")
                                v.tensor_scalar(out=m[:], in0=nw8[:],
                                                scalar1=float(t + 8),
                                                scalar2=None,
                                                op0=AL.is_equal)
                                mw[t] = m
                            mv = {}
                            for s in TS:
                                m = wk.tile([128, F], I16, tag=f"mv{s}")
                                v.tensor_scalar(out=m[:], in0=kh8[:],
                                                scalar1=float(s + 8),
                                                scalar2=None,
                                                op0=AL.is_equal)
                                mv[s] = m

                            # fractions (GPSIMD)
                            flw = wk.tile([128, F], FP16, tag="flw")
                            g.tensor_scalar(out=flw[:], in0=nw8[:],
                                            scalar1=8.0, scalar2=None,
                                            op0=AL.subtract)
                            gam = wk.tile([128, F], FP16, tag="gam")
                            g.tensor_tensor(out=gam[:], in0=dw16[:],
                                            in1=flw[:], op=AL.subtract)
                            flh = wk.tile([128, F], FP16, tag="flh")
                            g.tensor_scalar(out=flh[:], in0=kh8[:],
                                            scalar1=8.0, scalar2=None,
                                            op0=AL.subtract)
                            alp = wk.tile([128, F], FP16, tag="alp")
                            g.tensor_tensor(out=alp[:], in0=dh16[:],
                                            in1=flh[:], op=AL.subtract)

                            # horizontal one-hot: all 9 rows in one 3D op.
                            # init comes straight from DRAM (shift -1 slice)
                            # instead of a slow strided SBUF copy.
                            cb = wk.tile([128, 9 * F], I32, tag="cb")
                            cb9 = cb[:].rearrange("p (r c) -> p r c", r=9)
                            for r in range(9):
                                nc.sync.dma_start(
                                    out=cb9[:, r:r + 1, :],
                                    in_=hp3[b, PAD + r0 - 4 + r:
                                            PAD + r0 - 4 + r + 128,
                                            PAD + w0 + INIT:
                                            PAD + w0 + INIT + F].unsqueeze(1))
                            for t in TS:
                                mb = mw[t][:].unsqueeze(1).broadcast_to(
                                    (128, 9, F))
                                v.copy_predicated(
                                    out=cb9[:], mask=mb,
                                    data=hp9[:, :, t + 4:t + 4 + F])

                            # vertical one-hot, N and S fused: [128, 2, F]
                            ns = wk.tile([128, 2 * F], I32, tag="ns")
                            v.tensor_copy(out=ns[:, 0:F],
                                          in_=cb[:, (INIT + 4) * F:
                                                 (INIT + 5) * F])
                            v.tensor_copy(out=ns[:, F:2 * F],
                                          in_=cb[:, (INIT + 5) * F:
                                                 (INIT + 6) * F])
                            ns2 = ns[:].rearrange("p (r c) -> p r c", r=2)
                            for s in TS:
                                mb = mv[s][:].unsqueeze(1).broadcast_to(
                                    (128, 2, F))
                                v.copy_predicated(
                                    out=ns2[:], mask=mb,
                                    data=cb9[:, s + 4:s + 6, :])

                            # unpack + bilinear lerp (GPSIMD)
                            nsv = ns[:].bitcast(FP16).rearrange(
                                "p (r c two) -> p r c two", r=2, two=2)
                            nw_v = nsv[:, 0, :, 0:1].squeeze(-1)
                            ne_v = nsv[:, 0, :, 1:2].squeeze(-1)
                            sw_v = nsv[:, 1, :, 0:1].squeeze(-1)
                            se_v = nsv[:, 1, :, 1:2].squeeze(-1)

                            dn = wk.tile([128, F], FP16, tag="dn")
                            g.tensor_tensor(out=dn[:], in0=ne_v, in1=nw_v,
                                            op=AL.subtract)
                            t1 = wk.tile([128, F], FP16, tag="t1")
                            g.tensor_tensor(out=t1[:], in0=gam[:], in1=dn[:],
                                            op=AL.mult)
                            hn = wk.tile([128, F], FP16, tag="hn")
                            g.tensor_tensor(out=hn[:], in0=nw_v, in1=t1[:],
                                            op=AL.add)
                            ds = wk.tile([128, F], FP16, tag="ds")
                            g.tensor_tensor(out=ds[:], in0=se_v, in1=sw_v,
                                            op=AL.subtract)
                            t2 = wk.tile([128, F], FP16, tag="t2")
                            g.tensor_tensor(out=t2[:], in0=gam[:], in1=ds[:],
                                            op=AL.mult)
                            hs = wk.tile([128, F], FP16, tag="hs")
                            g.tensor_tensor(out=hs[:], in0=sw_v, in1=t2[:],
                                            op=AL.add)
                            dv_ = wk.tile([128, F], FP16, tag="dv")
                            g.tensor_tensor(out=dv_[:], in0=hs[:], in1=hn[:],
                                            op=AL.subtract)
                            t3 = wk.tile([128, F], FP16, tag="t3")
                            g.tensor_tensor(out=t3[:], in0=alp[:], in1=dv_[:],
                                            op=AL.mult)
                            o16 = wk.tile([128, F], FP16, tag="o16")
                            g.tensor_tensor(out=o16[:], in0=hn[:], in1=t3[:],
                                            op=AL.add)
                            nc.sync.dma_start(
                                out=out3[b, r0:r0 + 128, w0:w0 + F],
                                in_=o16[:])

            # ---- fixup scatter (after dense stores) ----
            for c in range(NCH):
                g.indirect_dma_start(
                    out=outf[:, None],
                    out_offset=IndirectOffsetOnAxis(
                        ap=opos_s[:, c:c + 1], axis=0),
                    in_=acc16[:, c:c + 1], in_offset=None)

    nc.compile()
    return nc


_PROGRAM_CACHE = {}


def _get_program(nout):
    if nout not in _PROGRAM_CACHE:
        _PROGRAM_CACHE[nout] = _build_program(nout)
    return _PROGRAM_CACHE[nout]


def _host_inlier_mask(d):
    """Mirror the device fp16 floor: rint(fp32(fp16(d)) + 7.5) in [4, 11]."""
    d16 = d.astype(np.float16).astype(np.float32)
    n8 = np.rint(d16 + np.float32(7.5)).astype(np.int32)
    return (n8 >= S_LO + 8) & (n8 <= S_HI + 8)


def _host_metadata(dH, dW):
    """Outlier positions for one image under the device dense criterion."""
    inl = _host_inlier_mask(dH) & _host_inlier_mask(dW)
    oy, ox = np.where(~inl)
    return oy.astype(np.int64), ox.astype(np.int64)


def _prepare(input1, input2):
    """Build (or fetch) the program and the per-core input maps."""
    input1 = np.asarray(input1)
    input2 = np.asarray(input2)
    assert input1.shape == (B, 1, H, W) and input2.shape == (B, 2, H, W)

    metas = []
    max_n = 1
    for c in range(NCORES):
        rows = []
        for bl in range(BPC):
            bglob = c * BPC + bl
            oy, ox = _host_metadata(input2[bglob, 0], input2[bglob, 1])
            rows.append((bl, oy, ox))
        n = sum(len(oy) for _, oy, _ in rows)
        max_n = max(max_n, n)
        metas.append(rows)
    nout = max(128, ((max_n + 127) // 128) * 128)

    nc = _get_program(nout)

    in_maps = []
    for c in range(NCORES):
        imgs = input1[c * BPC:(c + 1) * BPC, 0]
        flow = input2[c * BPC:(c + 1) * BPC]
        opos = np.full(nout, BPC * HW, np.int32)
        odh = np.zeros(nout, np.int32)
        odw = np.full(nout, HW, np.int32)
        oh = np.zeros(nout, f32)
        ow = np.zeros(nout, f32)
        obase = np.zeros(nout, f32)
        k = 0
        for bl, oy, ox in metas[c]:
            n = len(oy)
            opos[k:k + n] = (bl * HW + oy * W + ox).astype(np.int32)
            odh[k:k + n] = (bl * 2 * HW + oy * W + ox).astype(np.int32)
            odw[k:k + n] = (bl * 2 * HW + HW + oy * W + ox).astype(np.int32)
            oh[k:k + n] = oy.astype(f32)
            ow[k:k + n] = ox.astype(f32)
            obase[k:k + n] = f32(bl * PP * PP)
            k += n
        in_maps.append({
            "img": np.ascontiguousarray(imgs),
            "flow": np.ascontiguousarray(flow.reshape(-1)),
            "opos": opos, "odh": odh, "odw": odw,
            "oh": oh, "oh1": (oh + f32(1.0)).astype(f32),
            "ow": ow, "ow1": (ow + f32(1.0)).astype(f32),
            "obase": obase,
        })

    return nc, in_maps


def _assemble(results):
    out = np.empty((B, 1, H, W), f32)
    for c in range(NCORES):
        o = results[c]["out"][:BPC * HW].reshape(BPC, H, W)
        out[c * BPC:(c + 1) * BPC, 0] = o.astype(f32)
    return out


def kernel(input1, input2):
    nc, in_maps = _prepare(input1, input2)
    res = run_bass_kernel_spmd(nc, in_maps, core_ids=list(range(NCORES)))
    return _assemble(res.results)


# revision 9
# speedup vs baseline: 1.3169x; 1.3047x over previous
"""Dense2DSpatialTransformer (bilinear warp, N(0,1) flow) on 8 TRN2 cores.

V4 design. The per-pixel 2D gather runs as one-hot copy_predicated selects,
with every select element carrying TWO fp16 pixels packed in one int32:

  * Phase 0 builds an int32 "HP" image in DRAM directly from the input
    (element c = packed fp16 pair (I[h,c], I[h,c+1])), with replicate
    padding done in-SBUF (cols) and via 1-row DRAM copies (rows).  An fp32
    replicate-padded image is built in parallel for the exact sparse fixup.
  * Dense per tile: 9 candidate-row HP planes live in ONE SBUF tile; the
    8-way horizontal one-hot runs as 7 wide 3D-AP copy_predicated ops
    (mask broadcast over the row dim).  The two vertical one-hots (floor
    row N and ceil row S) are fused into one [128,2,F] select chain.
  * Masks/fractions from an fp16 flow copy via tensor_scalar (4x DVE mode);
    the DVE float->int convert rounds, so floors use a +7.5 bias.
  * Fractions + bilinear lerp are offloaded to the idle GPSIMD engine;
    fp32->fp16 flow converts to the Activation engine.  Output is fp16,
    widened on host.
  * Outliers (|disp| beyond the window, ~1e-4) are computed exactly in
    fp32 via indirect-DMA gathers (hoisted before the dense loop) and
    scattered over the dense result at the end.
"""
import sys

for _p in ("/opt/trn_rl_repo", "/opt/trn_rl_repo/concourse",
           "/root/.axon_site/_ro/trn_rl_repo"):
    if _p not in sys.path:
        sys.path.insert(0, _p)

import numpy as np

import concourse.bass as bass
import concourse.bacc as bacc
import concourse.mybir as mybir
import concourse.tile as tile
from concourse.bass import IndirectOffsetOnAxis
from concourse.bass_utils import run_bass_kernel_spmd

f32 = np.float32
FP = mybir.dt.float32
FP16 = mybir.dt.float16
I32 = mybir.dt.int32
I16 = mybir.dt.int16

B, H, W = 16, 1024, 1024
NCORES = 8
BPC = B // NCORES           # images per core
PAD = 8
PP = H + 2 * PAD            # padded image side (1040)
S_LO, S_HI = -4, 3          # dense integer-shift window (per axis)
F = 512                     # free-dim tile width
NROW = H // 128             # row blocks per image
NCOL = W // F               # col chunks per image
HW = H * W
OUT_TAIL = 128              # scratch tail for fixup padding writes
INIT = -1                   # shift covered by the one-hot init copy
HPW = 528                   # per-plane stride in the 9-plane HP mega-tile

AL = mybir.AluOpType


def _build_program(nout):
    nc = bacc.Bacc("TRN2", target_bir_lowering=False, debug=False,
                   enable_asserts=False, num_devices=NCORES)

    img_d = nc.dram_tensor("img", [BPC, H, W], FP, kind="ExternalInput")
    flow_d = nc.dram_tensor("flow", [BPC * 2 * HW], FP, kind="ExternalInput")
    opos_d = nc.dram_tensor("opos", [nout], I32, kind="ExternalInput")
    odh_d = nc.dram_tensor("odh", [nout], I32, kind="ExternalInput")
    odw_d = nc.dram_tensor("odw", [nout], I32, kind="ExternalInput")
    oh_d = nc.dram_tensor("oh", [nout], FP, kind="ExternalInput")
    oh1_d = nc.dram_tensor("oh1", [nout], FP, kind="ExternalInput")
    ow_d = nc.dram_tensor("ow", [nout], FP, kind="ExternalInput")
    ow1_d = nc.dram_tensor("ow1", [nout], FP, kind="ExternalInput")
    obase_d = nc.dram_tensor("obase", [nout], FP, kind="ExternalInput")
    ppad_d = nc.dram_tensor("ppad", [BPC * PP * PP], FP, kind="Internal")
    hp_d = nc.dram_tensor("hp", [BPC, PP, PP], I32, kind="Internal")
    out_d = nc.dram_tensor("out", [BPC * HW + OUT_TAIL], FP16,
                           kind="ExternalOutput")

    img = img_d.ap()
    flowf = flow_d.ap()
    flow4 = flowf.rearrange("(b c h w) -> b c h w", b=BPC, c=2, h=H, w=W)
    ppf = ppad_d.ap()
    pp3 = ppf.rearrange("(b h w) -> b h w", b=BPC, h=PP, w=PP)
    hp3 = hp_d.ap()
    outf = out_d.ap()
    out3 = outf[0:BPC * HW].rearrange("(b h w) -> b h w", b=BPC, h=H, w=W)

    v = nc.vector
    g = nc.gpsimd

    with tile.TileContext(nc) as tc:
        # ---- phase 0a: HP packed-pair fp16 image, directly from img ----
        # hp[b, 8+h, c] = int32( fp16(I[h,c-8]), fp16(I[h,c-8+1]) ), with
        # replicate padding: cols done in-SBUF, rows via 1-row DRAM copies.
        with tc.tile_pool(name="hpb", bufs=2) as hb:
            for b in range(BPC):
                for blk in range(NROW):
                    rs = 128 * blk
                    p32 = hb.tile([128, PP], FP, tag="p32")
                    nc.sync.dma_start(out=p32[:, PAD:PAD + W],
                                      in_=img[b, rs:rs + 128, :])
                    v.tensor_copy(
                        out=p32[:, 3:PAD],
                        in_=p32[:, PAD:PAD + 1].broadcast_to((128, PAD - 3)))
                    v.tensor_copy(
                        out=p32[:, PAD + W:PAD + W + 6],
                        in_=p32[:, PAD + W - 1:PAD + W].broadcast_to((128, 6)))
                    hpt = hb.tile([128, PP], I32, tag="hpt")
                    v16 = hpt[:].bitcast(FP16).rearrange(
                        "p (c two) -> p c two", two=2)
                    v.tensor_copy(out=v16[:, 3:1037, 0:1],
                                  in_=p32[:, 3:1037].unsqueeze(-1))
                    nc.scalar.copy(out=v16[:, 3:1037, 1:2],
                                   in_=p32[:, 4:1038].unsqueeze(-1))
                    nc.sync.dma_start(out=hp3[b, PAD + rs:PAD + rs + 128,
                                              3:1037],
                                      in_=hpt[:, 3:1037])
                # replicate packed edge rows (row 8 -> 3..7, 1031 -> 1032..36)
                for k in range(3, PAD):
                    nc.sync.dma_start(out=hp3[b, k:k + 1, 3:1037],
                                      in_=hp3[b, PAD:PAD + 1, 3:1037])
                for k in range(PAD + H, PAD + H + 5):
                    nc.sync.dma_start(out=hp3[b, k:k + 1, 3:1037],
                                      in_=hp3[b, PAD + H - 1:PAD + H, 3:1037])

        # ---- phase 0b: fp32 replicate-padded image (fixup gathers) ----
        for b in range(BPC):
            nc.sync.dma_start(out=pp3[b, PAD:PAD + H, PAD:PAD + W],
                              in_=img[b])
            for k in range(PAD):
                nc.sync.dma_start(out=pp3[b, k:k + 1, PAD:PAD + W],
                                  in_=img[b, 0:1, :])
                nc.sync.dma_start(
                    out=pp3[b, PAD + H + k:PAD + H + k + 1, PAD:PAD + W],
                    in_=img[b, H - 1:H, :])
        rblocks = []
        rs = 0
        while rs < PP:
            rn = min(128, PP - rs)
            rblocks.append((rs, rn))
            rs += rn
        with tc.tile_pool(name="pad", bufs=2) as pd:
            for b in range(BPC):
                for (rs, rn) in rblocks:
                    cl = pd.tile([128, 1], FP, tag="cl")
                    nc.sync.dma_start(out=cl[0:rn],
                                      in_=pp3[b, rs:rs + rn, PAD:PAD + 1])
                    ct = pd.tile([128, PAD], FP, tag="ct")
                    v.tensor_copy(out=ct[0:rn],
                                  in_=cl[0:rn, 0:1].broadcast_to((rn, PAD)))
                    nc.sync.dma_start(out=pp3[b, rs:rs + rn, 0:PAD],
                                      in_=ct[0:rn])
                    cr = pd.tile([128, 1], FP, tag="cr")
                    nc.sync.dma_start(
                        out=cr[0:rn],
                        in_=pp3[b, rs:rs + rn, PAD + W - 1:PAD + W])
                    cu = pd.tile([128, PAD], FP, tag="cu")
                    v.tensor_copy(out=cu[0:rn],
                                  in_=cr[0:rn, 0:1].broadcast_to((rn, PAD)))
                    nc.sync.dma_start(
                        out=pp3[b, rs:rs + rn, PAD + W:PAD + W + PAD],
                        in_=cu[0:rn])

        TS = [t for t in range(S_LO, S_HI + 1) if t != INIT]
        NCH = nout // 128

        # ---- sparse fixup, compute part (hoisted before the dense loop) ----
        with tc.tile_pool(name="fix", bufs=1) as fx:
            def load_aux(d, dt, name):
                t = fx.tile([128, NCH], dt, tag=name)
                nc.sync.dma_start(
                    out=t[:],
                    in_=d.ap().rearrange("(p f) -> p f", p=128))
                return t

            opos_s = load_aux(opos_d, I32, "opos")
            odh_s = load_aux(odh_d, I32, "odh")
            odw_s = load_aux(odw_d, I32, "odw")
            oh_s = load_aux(oh_d, FP, "oh")
            oh1_s = load_aux(oh1_d, FP, "oh1")
            ow_s = load_aux(ow_d, FP, "ow")
            ow1_s = load_aux(ow1_d, FP, "ow1")
            obase_s = load_aux(obase_d, FP, "obase")

            dhv = fx.tile([128, NCH], FP, tag="dhv")
            dwv = fx.tile([128, NCH], FP, tag="dwv")
            for c in range(NCH):
                g.indirect_dma_start(
                    out=dhv[:, c:c + 1], out_offset=None,
                    in_=flowf[:, None],
                    in_offset=IndirectOffsetOnAxis(
                        ap=odh_s[:, c:c + 1], axis=0))
                g.indirect_dma_start(
                    out=dwv[:, c:c + 1], out_offset=None,
                    in_=flowf[:, None],
                    in_offset=IndirectOffsetOnAxis(
                        ap=odw_s[:, c:c + 1], axis=0))

            def fields(dv, hb_, hb1, pfx):
                yt = fx.tile([128, NCH], FP, tag=f"{pfx}y")
                v.tensor_tensor(out=yt[:], in0=dv[:], in1=hb_[:], op=AL.add)
                v.tensor_scalar(out=yt[:], in0=yt[:], scalar1=1.0,
                                scalar2=None, op0=AL.add)
                Rt = fx.tile([128, NCH], FP, tag=f"{pfx}R")
                v.tensor_tensor(out=Rt[:], in0=yt[:], in1=hb1[:],
                                op=AL.subtract)
                St = fx.tile([128, NCH], FP, tag=f"{pfx}S")
                gt = fx.tile([128, NCH], FP, tag=f"{pfx}g")
                v.tensor_scalar(out=St[:], in0=Rt[:], scalar1=-6.0,
                                scalar2=None, op0=AL.is_ge)
                for s in range(-5, 7):
                    v.tensor_scalar(out=gt[:], in0=Rt[:], scalar1=float(s),
                                    scalar2=None, op0=AL.is_ge)
                    v.tensor_tensor(out=St[:], in0=St[:], in1=gt[:],
                                    op=AL.add)
                dt_ = fx.tile([128, NCH], FP, tag=f"{pfx}d")
                v.tensor_scalar(out=dt_[:], in0=St[:], scalar1=-6.0,
                                scalar2=None, op0=AL.add)
                v.tensor_tensor(out=dt_[:], in0=dt_[:], in1=Rt[:],
                                op=AL.subtract)
                return yt, dt_

            yv, dhw = fields(dhv, oh_s, oh1_s, "fh")
            ywv, dww = fields(dwv, ow_s, ow1_s, "fw")

            rowp = fx.tile([128, NCH], FP, tag="rowp")
            v.tensor_tensor(out=rowp[:], in0=yv[:], in1=dhw[:], op=AL.add)
            v.tensor_scalar(out=rowp[:], in0=rowp[:], scalar1=6.0,
                            scalar2=float(PP), op0=AL.add, op1=AL.mult)
            colp = fx.tile([128, NCH], FP, tag="colp")
            v.tensor_tensor(out=colp[:], in0=ywv[:], in1=dww[:], op=AL.add)
            v.tensor_scalar(out=colp[:], in0=colp[:], scalar1=6.0,
                            scalar2=None, op0=AL.add)
            af = fx.tile([128, NCH], FP, tag="af")
            v.tensor_tensor(out=af[:], in0=rowp[:], in1=colp[:], op=AL.add)
            v.tensor_tensor(out=af[:], in0=af[:], in1=obase_s[:], op=AL.add)

            vals = {}
            afo = fx.tile([128, NCH], FP, tag="afo")
            for (cn, doff) in (("v00", 0.0), ("v10", 1.0),
                               ("v01", float(PP)), ("v11", float(PP + 1))):
                ai = fx.tile([128, NCH], I32, tag=f"ai{cn}")
                if doff == 0.0:
                    v.tensor_copy(out=ai[:], in_=af[:])
                else:
                    v.tensor_scalar(out=afo[:], in0=af[:], scalar1=doff,
                                    scalar2=None, op0=AL.add)
                    v.tensor_copy(out=ai[:], in_=afo[:])
                vt = fx.tile([128, NCH], FP, tag=cn)
                for c in range(NCH):
                    g.indirect_dma_start(
                        out=vt[:, c:c + 1], out_offset=None,
                        in_=ppf[:, None],
                        in_offset=IndirectOffsetOnAxis(
                            ap=ai[:, c:c + 1], axis=0))
                vals[cn] = vt

            omw_f = fx.tile([128, NCH], FP, tag="omwf")
            v.tensor_scalar(out=omw_f[:], in0=dww[:], scalar1=-1.0,
                            scalar2=1.0, op0=AL.mult, op1=AL.add)
            omh_f = fx.tile([128, NCH], FP, tag="omhf")
            v.tensor_scalar(out=omh_f[:], in0=dhw[:], scalar1=-1.0,
                            scalar2=1.0, op0=AL.mult, op1=AL.add)
            wt = fx.tile([128, NCH], FP, tag="wtf")
            accf = fx.tile([128, NCH], FP, tag="accf")
            t3f = fx.tile([128, NCH], FP, tag="t3f")
            v.tensor_tensor(out=wt[:], in0=dhw[:], in1=dww[:], op=AL.mult)
            v.tensor_tensor(out=accf[:], in0=vals["v00"][:], in1=wt[:],
                            op=AL.mult)
            v.tensor_tensor(out=wt[:], in0=dhw[:], in1=omw_f[:], op=AL.mult)
            v.tensor_tensor(out=t3f[:], in0=vals["v10"][:], in1=wt[:],
                            op=AL.mult)
            v.tensor_tensor(out=accf[:], in0=accf[:], in1=t3f[:], op=AL.add)
            v.tensor_tensor(out=wt[:], in0=omh_f[:], in1=dww[:], op=AL.mult)
            v.tensor_tensor(out=t3f[:], in0=vals["v01"][:], in1=wt[:],
                            op=AL.mult)
            v.tensor_tensor(out=accf[:], in0=accf[:], in1=t3f[:], op=AL.add)
            v.tensor_tensor(out=wt[:], in0=omw_f[:], in1=omh_f[:],
                            op=AL.mult)
            v.tensor_tensor(out=t3f[:], in0=vals["v11"][:], in1=wt[:],
                            op=AL.mult)
            v.tensor_tensor(out=accf[:], in0=accf[:], in1=t3f[:], op=AL.add)
            acc16 = fx.tile([128, NCH], FP16, tag="acc16")
            v.tensor_copy(out=acc16[:], in_=accf[:])

            # ---- dense tiles ----
            with tc.tile_pool(name="wk", bufs=2) as wk:
                for b in range(BPC):
                    for i in range(NROW):
                        for j in range(NCOL):
                            r0 = 128 * i
                            w0 = F * j
                            hpa = wk.tile([128, 9 * HPW], I32, tag="hpa")
                            hp9 = hpa[:].rearrange("p (r c) -> p r c", r=9)
                            for r in range(9):
                                nc.sync.dma_start(
                                    out=hp9[:, r:r + 1, 0:F + 8],
                                    in_=hp3[b, PAD + r0 - 4 + r:
                                            PAD + r0 - 4 + r + 128,
                                            PAD + w0 - 4:PAD + w0 + F + 4
                                            ].unsqueeze(1))
                            f32h = wk.tile([128, F], FP, tag="f32h")
                            nc.sync.dma_start(
                                out=f32h[:],
                                in_=flow4[b, 0, r0:r0 + 128, w0:w0 + F])
                            f32w = wk.tile([128, F], FP, tag="f32w")
                            nc.sync.dma_start(
                                out=f32w[:],
                                in_=flow4[b, 1, r0:r0 + 128, w0:w0 + F])
                            dh16 = wk.tile([128, F], FP16, tag="dh16")
                            nc.scalar.copy(out=dh16[:], in_=f32h[:])
                            dw16 = wk.tile([128, F], FP16, tag="dw16")
                            nc.scalar.copy(out=dw16[:], in_=f32w[:])

                            # floors (DVE float->int convert ROUNDS: +7.5)
                            nw8 = wk.tile([128, F], I16, tag="nw8")
                            v.tensor_scalar(out=nw8[:], in0=dw16[:],
                                            scalar1=7.5, scalar2=None,
                                            op0=AL.add)
                            kh8 = wk.tile([128, F], I16, tag="kh8")
                            v.tensor_scalar(out=kh8[:], in0=dh16[:],
                                            scalar1=7.5, scalar2=None,
                                            op0=AL.add)

                            # int16-input DVE ops are slow; convert the floor
                            # codes back to fp16 once and stay in the fast
                            # fp16-input class for masks and fractions
                            nwf = wk.tile([128, F], FP16, tag="nwf")
                            v.tensor_copy(out=nwf[:], in_=nw8[:])
                            khf = wk.tile([128, F], FP16, tag="khf")
                            v.tensor_copy(out=khf[:], in_=kh8[:])

                            mw = {}
                            for t in TS:
                                m = wk.tile([128, F], I16, tag=f"mw{t}")
                                v.tensor_scalar(out=m[:], in0=nwf[:],
                                                scalar1=float(t + 8),
                                                scalar2=None,
                                                op0=AL.is_equal)
                                mw[t] = m
                            mv = {}
                            for s in TS:
                                m = wk.tile([128, F], I16, tag=f"mv{s}")
                                v.tensor_scalar(out=m[:], in0=khf[:],
                                                scalar1=float(s + 8),
                                                scalar2=None,
                                                op0=AL.is_equal)
                                mv[s] = m

                            # fractions (fast fp16 class, DVE)
                            flw = wk.tile([128, F], FP16, tag="flw")
                            v.tensor_scalar(out=flw[:], in0=nwf[:],
                                            scalar1=8.0, scalar2=None,
                                            op0=AL.subtract)
                            gam = wk.tile([128, F], FP16, tag="gam")
                            v.tensor_tensor(out=gam[:], in0=dw16[:],
                                            in1=flw[:], op=AL.subtract)
                            flh = wk.tile([128, F], FP16, tag="flh")
                            v.tensor_scalar(out=flh[:], in0=khf[:],
                                            scalar1=8.0, scalar2=None,
                                            op0=AL.subtract)
                            alp = wk.tile([128, F], FP16, tag="alp")
                            v.tensor_tensor(out=alp[:], in0=dh16[:],
                                            in1=flh[:], op=AL.subtract)

                            # horizontal one-hot: all 9 rows in one 3D op.
                            # init comes straight from DRAM (shift -1 slice)
                            # instead of a slow strided SBUF copy.
                            cb = wk.tile([128, 9 * F], I32, tag="cb")
                            cb9 = cb[:].rearrange("p (r c) -> p r c", r=9)
                            for r in range(9):
                                nc.sync.dma_start(
                                    out=cb9[:, r:r + 1, :],
                                    in_=hp3[b, PAD + r0 - 4 + r:
                                            PAD + r0 - 4 + r + 128,
                                            PAD + w0 + INIT:
                                            PAD + w0 + INIT + F].unsqueeze(1))
                            for t in TS:
                                mb = mw[t][:].unsqueeze(1).broadcast_to(
                                    (128, 9, F))
                                v.copy_predicated(
                                    out=cb9[:], mask=mb,
                                    data=hp9[:, :, t + 4:t + 4 + F])

                            # vertical one-hot, N and S fused: [128, 2, F]
                            ns = wk.tile([128, 2 * F], I32, tag="ns")
                            v.tensor_copy(out=ns[:, 0:F],
                                          in_=cb[:, (INIT + 4) * F:
                                                 (INIT + 5) * F])
                            v.tensor_copy(out=ns[:, F:2 * F],
                                          in_=cb[:, (INIT + 5) * F:
                                                 (INIT + 6) * F])
                            ns2 = ns[:].rearrange("p (r c) -> p r c", r=2)
                            for s in TS:
                                mb = mv[s][:].unsqueeze(1).broadcast_to(
                                    (128, 2, F))
                                v.copy_predicated(
                                    out=ns2[:], mask=mb,
                                    data=cb9[:, s + 4:s + 6, :])

                            # unpack + bilinear lerp (GPSIMD)
                            nsv = ns[:].bitcast(FP16).rearrange(
                                "p (r c two) -> p r c two", r=2, two=2)
                            nw_v = nsv[:, 0, :, 0:1].squeeze(-1)
                            ne_v = nsv[:, 0, :, 1:2].squeeze(-1)
                            sw_v = nsv[:, 1, :, 0:1].squeeze(-1)
                            se_v = nsv[:, 1, :, 1:2].squeeze(-1)

                            dn = wk.tile([128, F], FP16, tag="dn")
                            v.tensor_tensor(out=dn[:], in0=ne_v, in1=nw_v,
                                            op=AL.subtract)
                            t1 = wk.tile([128, F], FP16, tag="t1")
                            v.tensor_tensor(out=t1[:], in0=gam[:], in1=dn[:],
                                            op=AL.mult)
                            hn = wk.tile([128, F], FP16, tag="hn")
                            v.tensor_tensor(out=hn[:], in0=nw_v, in1=t1[:],
                                            op=AL.add)
                            ds = wk.tile([128, F], FP16, tag="ds")
                            v.tensor_tensor(out=ds[:], in0=se_v, in1=sw_v,
                                            op=AL.subtract)
                            t2 = wk.tile([128, F], FP16, tag="t2")
                            v.tensor_tensor(out=t2[:], in0=gam[:], in1=ds[:],
                                            op=AL.mult)
                            hs = wk.tile([128, F], FP16, tag="hs")
                            v.tensor_tensor(out=hs[:], in0=sw_v, in1=t2[:],
                                            op=AL.add)
                            dv_ = wk.tile([128, F], FP16, tag="dv")
                            v.tensor_tensor(out=dv_[:], in0=hs[:], in1=hn[:],
                                            op=AL.subtract)
                            t3 = wk.tile([128, F], FP16, tag="t3")
                            v.tensor_tensor(out=t3[:], in0=alp[:], in1=dv_[:],
                                            op=AL.mult)
                            o16 = wk.tile([128, F], FP16, tag="o16")
                            v.tensor_tensor(out=o16[:], in0=hn[:], in1=t3[:],
                                            op=AL.add)
                            nc.sync.dma_start(
                                out=out3[b, r0:r0 + 128, w0:w0 + F],
                                in_=o16[:])

            # ---- fixup scatter (after dense stores) ----
            for c in range(NCH):
                g.indirect_dma_start(
                    out=outf[:, None],
                    out_offset=IndirectOffsetOnAxis(
                        ap=opos_s[:, c:c + 1], axis=0),
                    in_=acc16[:, c:c + 1], in_offset=None)

    nc.compile()
    return nc


_PROGRAM_CACHE = {}


def _get_program(nout):
    if nout not in _PROGRAM_CACHE:
        _PROGRAM_CACHE[nout] = _build_program(nout)
    return _PROGRAM_CACHE[nout]


def _host_inlier_mask(d):
    """Mirror the device fp16 floor: rint(fp32(fp16(d)) + 7.5) in [4, 11]."""
    d16 = d.astype(np.float16).astype(np.float32)
    n8 = np.rint(d16 + np.float32(7.5)).astype(np.int32)
    return (n8 >= S_LO + 8) & (n8 <= S_HI + 8)


def _host_metadata(dH, dW):
    """Outlier positions for one image under the device dense criterion."""
    inl = _host_inlier_mask(dH) & _host_inlier_mask(dW)
    oy, ox = np.where(~inl)
    return oy.astype(np.int64), ox.astype(np.int64)


def _prepare(input1, input2):
    """Build (or fetch) the program and the per-core input maps."""
    input1 = np.asarray(input1)
    input2 = np.asarray(input2)
    assert input1.shape == (B, 1, H, W) and input2.shape == (B, 2, H, W)

    metas = []
    max_n = 1
    for c in range(NCORES):
        rows = []
        for bl in range(BPC):
            bglob = c * BPC + bl
            oy, ox = _host_metadata(input2[bglob, 0], input2[bglob, 1])
            rows.append((bl, oy, ox))
        n = sum(len(oy) for _, oy, _ in rows)
        max_n = max(max_n, n)
        metas.append(rows)
    nout = max(128, ((max_n + 127) // 128) * 128)

    nc = _get_program(nout)

    in_maps = []
    for c in range(NCORES):
        imgs = input1[c * BPC:(c + 1) * BPC, 0]
        flow = input2[c * BPC:(c + 1) * BPC]
        opos = np.full(nout, BPC * HW, np.int32)
        odh = np.zeros(nout, np.int32)
        odw = np.full(nout, HW, np.int32)
        oh = np.zeros(nout, f32)
        ow = np.zeros(nout, f32)
        obase = np.zeros(nout, f32)
        k = 0
        for bl, oy, ox in metas[c]:
            n = len(oy)
            opos[k:k + n] = (bl * HW + oy * W + ox).astype(np.int32)
            odh[k:k + n] = (bl * 2 * HW + oy * W + ox).astype(np.int32)
            odw[k:k + n] = (bl * 2 * HW + HW + oy * W + ox).astype(np.int32)
            oh[k:k + n] = oy.astype(f32)
            ow[k:k + n] = ox.astype(f32)
            obase[k:k + n] = f32(bl * PP * PP)
            k += n
        in_maps.append({
            "img": np.ascontiguousarray(imgs),
            "flow": np.ascontiguousarray(flow.reshape(-1)),
            "opos": opos, "odh": odh, "odw": odw,
            "oh": oh, "oh1": (oh + f32(1.0)).astype(f32),
            "ow": ow, "ow1": (ow + f32(1.0)).astype(f32),
            "obase": obase,
        })

    return nc, in_maps


def _assemble(results):
    out = np.empty((B, 1, H, W), f32)
    for c in range(NCORES):
        o = results[c]["out"][:BPC * HW].reshape(BPC, H, W)
        out[c * BPC:(c + 1) * BPC, 0] = o.astype(f32)
    return out


def kernel(input1, input2):
    nc, in_maps = _prepare(input1, input2)
    res = run_bass_kernel_spmd(nc, in_maps, core_ids=list(range(NCORES)))
    return _assemble(res.results)
